# revision 5
# baseline (speedup 1.0000x reference)
"""Bayesian block-sparse linear layer (gnn message passing) on 8 Trainium2 cores.

out = segment_sum_e( v[e].T @ x_block[col_g[e]] ) + bias,
v[e] = eps_w[e] * exp(weight_log_var[e]) + weight_mean[e]   (32x32 blocks)

Strategy:
  * Batch-shard x across the 8 cores (128 columns each) -> one SPMD program.
  * The graph (row_g/col_g) is host-visible, so the program is specialized to
    it: every per-edge matmul is a 32x32 PE sub-array matmul (tile_position)
    so up to 16 edge-matmuls execute concurrently on the PE array.
  * All of x lives in SBUF in a block-permuted layout (block c at partition
    group c%4).  Edge weights are sampled on device (ACT exp + 2 DVE ops)
    from host-packed per-round layouts, then consumed as matmul lhsT.
  * Output blocks are processed in 16 rounds of 16 blocks; each block owns a
    [32, 512] PSUM strip = 4 private accumulator copies (one per PE row
    group) so concurrent sub-array matmuls never accumulate into the same
    PSUM element.  Evacuation sums the 4 copies and adds the sampled bias
    (ACT Identity with per-partition bias + 3 DVE adds).
"""

import os

import numpy as np

# problem dims (hardcoded per spec)
G1 = 256
G2 = 256
A1 = 32
A2 = 32
B = 1024
NCORES = 8
BSHARD = B // NCORES  # 128
NROUNDS = 16
BPR = 16    # blocks per round
NBANKS = 4  # PSUM banks per round
ZED = object()  # sentinel

LAST_PROFILE = None
_prog_cache = {}


def _dt_mode():
    return os.environ.get("BSL_DTYPE", "fp32")


# ---------------------------------------------------------------- host plan

def _plan(row_g, col_g):
    """Specialize the schedule to the graph."""
    E = len(row_g)
    blk = [[[] for _ in range(4)] for _ in range(G2)]
    for e in range(E):
        blk[int(row_g[e])][int(col_g[e]) % 4].append(e)
    cnts = np.array([[len(blk[q][g]) for g in range(4)] for q in range(G2)])

    # greedy bin-pack blocks into rounds, balancing per-partition-group load
    order = np.argsort(-cnts.sum(1), kind="stable")
    rounds = [[] for _ in range(NROUNDS)]
    load = np.zeros((NROUNDS, 4), np.int64)
    for q in order:
        best, bkey = None, None
        for r in range(NROUNDS):
            if len(rounds[r]) >= BPR:
                continue
            nl = load[r] + cnts[q]
            key = (int(nl.max()), int(nl.sum()))
            if best is None or key < bkey:
                best, bkey = r, key
        rounds[best].append(int(q))
        load[best] += cnts[q]

    blkmap = {}
    for r in range(NROUNDS):
        for idx, q in enumerate(rounds[r]):
            blkmap[q] = (r, idx)

    sched = []
    for r in range(NROUNDS):
        glists = [[] for _ in range(4)]
        for q in rounds[r]:
            for g in range(4):
                lst = blk[q][g]
                if lst:
                    for e in lst:
                        glists[g].append((e, q))
                else:
                    # region never written otherwise -> dummy zero matmul
                    glists[g].append((E, q))
        # round-robin by slot so consecutive same-group matmuls hit
        # different PE sub-arrays
        for g in range(4):
            byslot = [[], [], [], []]
            for e, q in glists[g]:
                byslot[blkmap[q][1] // 4].append((e, q))
            inter = []
            i = 0
            while any(byslot):
                sl = byslot[i % 4]
                if sl:
                    inter.append(sl.pop(0))
                i += 1
            glists[g] = inter
        L = max(len(x) for x in glists)
        q0 = rounds[r][0]
        for g in range(4):
            while len(glists[g]) < L:
                glists[g].append((E, q0))
        # start/stop flags per (row-group bank g, slot s): the first matmul
        # on tile (g, s) clears that bank's 2KB slot row, the last closes
        # the accumulation group.  Only tile (g, s) writes that region, so
        # the clear/accumulate ordering is the PE's own FIFO - race-free.
        first_pos, last_pos = {}, {}
        for p in range(L):
            for g in range(4):
                e, q = glists[g][p]
                s = blkmap[q][1] // 4
                if (g, s) not in first_pos:
                    first_pos[(g, s)] = p
                last_pos[(g, s)] = p
        entries = [[None] * L for _ in range(4)]
        widx = np.full((4, L), E, np.int64)
        for p in range(L):
            for g in range(4):
                e, q = glists[g][p]
                ridx = blkmap[q][1]
                s, u = ridx // 4, ridx % 4
                xcol = (int(col_g[e]) // 4) if e < E else 0
                entries[g][p] = (s, u, xcol,
                                 first_pos[(g, s)] == p,
                                 last_pos[(g, s)] == p)
                widx[g, p] = e
        sched.append({"L": L, "entries": entries, "widx": widx})
    return sched, rounds


# ---------------------------------------------------------------- host pack

def _pack_weights(w, sched, np_dt):
    w = np.asarray(w, np.float32).reshape(-1, A1, A2)
    wext = np.concatenate([w, np.zeros((1, A1, A2), np.float32)], 0)
    outs = []
    for sc in sched:
        t = wext[sc["widx"]]                       # [4, L, 32, 32]
        t = t.transpose(0, 2, 1, 3).reshape(128, 32 * sc["L"])
        outs.append(np.ascontiguousarray(t.astype(np_dt)))
    return outs


def _pack_x(xk, np_dt):  # xk [8192, BSHARD]
    t = xk.reshape(64, 4, 32, BSHARD).transpose(1, 2, 0, 3)
    return np.ascontiguousarray(t.reshape(128, 64 * BSHARD).astype(np_dt))


def _pack_bias(vec, rounds):  # vec [8192] fp32
    out = np.zeros((NROUNDS, NBANKS, 128), np.float32)
    for r in range(NROUNDS):
        for idx, q in enumerate(rounds[r]):
            s, bb = idx // 4, idx % 4
            out[r, bb, 32 * s:32 * s + 32] = vec[32 * q:32 * q + 32]
    return np.ascontiguousarray(
        out.transpose(2, 0, 1).reshape(128, NROUNDS * NBANKS))


def _unpack_out(op, rounds):  # op [NROUNDS, 128, 4*BSHARD] fp32
    t = op.reshape(NROUNDS, 4, 32, 4, BSHARD).transpose(0, 1, 3, 2, 4)
    res = np.zeros((G2, 32, BSHARD), np.float32)
    for r in range(NROUNDS):
        k = t[r].reshape(16, 32, BSHARD)
        for idx, q in enumerate(rounds[r]):
            res[q] = k[idx]
    return res.reshape(G2 * 32, BSHARD)


# ---------------------------------------------------------------- program

def _build(sched, dt_w):
    import concourse.bacc as bacc
    import concourse.mybir as mybir
    import concourse.tile as tile_mod

    nc = bacc.Bacc("TRN2", target_bir_lowering=False, debug=False,
                   num_devices=NCORES)
    f32 = mybir.dt.float32
    AF = mybir.ActivationFunctionType
    ADD = mybir.AluOpType.add
    MUL = mybir.AluOpType.mult

    x_d = nc.dram_tensor("x_packed", [128, 64 * BSHARD], dt_w,
                         kind="ExternalInput")
    wm_d = [nc.dram_tensor(f"wm_{r}", [128, 32 * sched[r]["L"]], dt_w,
                           kind="ExternalInput") for r in range(NROUNDS)]
    wl_d = [nc.dram_tensor(f"wl_{r}", [128, 32 * sched[r]["L"]], dt_w,
                           kind="ExternalInput") for r in range(NROUNDS)]
    we_d = [nc.dram_tensor(f"we_{r}", [128, 32 * sched[r]["L"]], dt_w,
                           kind="ExternalInput") for r in range(NROUNDS)]
    bm_d = nc.dram_tensor("bm_packed", [128, NROUNDS * NBANKS], f32,
                          kind="ExternalInput")
    bl_d = nc.dram_tensor("bl_packed", [128, NROUNDS * NBANKS], f32,
                          kind="ExternalInput")
    be_d = nc.dram_tensor("be_packed", [128, NROUNDS * NBANKS], f32,
                          kind="ExternalInput")
    out_d = nc.dram_tensor("out_packed", [NROUNDS, 128, NBANKS * BSHARD], f32,
                           kind="ExternalOutput")

    with tile_mod.TileContext(nc) as tc:
        with tc.tile_pool(name="xp", bufs=1) as xp, \
             tc.tile_pool(name="bp", bufs=1) as bp, \
             tc.tile_pool(name="wp", bufs=2) as wp, \
             tc.tile_pool(name="pp", bufs=8, space="PSUM") as pp, \
             tc.tile_pool(name="ep", bufs=8) as ep, \
             tc.tile_pool(name="opool", bufs=2) as opool:

            xt = xp.tile([128, 64 * BSHARD], dt_w, name="xt")
            nc.sync.dma_start(xt[:, :], x_d.ap())

            bmt = bp.tile([128, NROUNDS * NBANKS], f32, name="bmt")
            nc.sync.dma_start(bmt[:, :], bm_d.ap())
            blt = bp.tile([128, NROUNDS * NBANKS], f32, name="blt")
            nc.sync.dma_start(blt[:, :], bl_d.ap())
            bet = bp.tile([128, NROUNDS * NBANKS], f32, name="bet")
            nc.sync.dma_start(bet[:, :], be_d.ap())
            bias = bp.tile([128, NROUNDS * NBANKS], f32, name="bias")
            nc.scalar.activation(bias[:, :], blt[:, :], AF.Exp)
            nc.vector.tensor_tensor(bias[:, :], bias[:, :], bet[:, :], MUL)
            nc.vector.tensor_tensor(bias[:, :], bias[:, :], bmt[:, :], ADD)

            for r in range(NROUNDS):
                L = sched[r]["L"]
                W = 32 * L
                wmt = wp.tile([128, W], dt_w, tag="wm", name=f"wm_t{r}")
                nc.sync.dma_start(wmt[:, :], wm_d[r].ap())
                wlt = wp.tile([128, W], dt_w, tag="wl", name=f"wl_t{r}")
                nc.sync.dma_start(wlt[:, :], wl_d[r].ap())
                wet = wp.tile([128, W], dt_w, tag="we", name=f"we_t{r}")
                nc.sync.dma_start(wet[:, :], we_d[r].ap())

                # v = eps * exp(log_var) + mean
                nc.scalar.activation(wlt[:, :], wlt[:, :], AF.Exp)
                nc.vector.tensor_tensor(wet[:, :], wet[:, :], wlt[:, :], MUL)
                nc.vector.tensor_tensor(wet[:, :], wet[:, :], wmt[:, :], ADD)

                # bank g is private to PE row-group g: no two sub-array
                # matmuls ever write the same (bank, partition-range).
                banks = [pp.tile([128, NBANKS * BSHARD], f32, tag="bank",
                                 name=f"bank{r}_{b}") for b in range(4)]
                ents = sched[r]["entries"]
                for p in range(L):
                    for g in range(4):
                        s, u, xcol, st, sp = ents[g][p]
                        nc.tensor.matmul(
                            banks[g][32 * s:32 * s + 32,
                                     BSHARD * u:BSHARD * u + BSHARD],
                            lhsT=wet[32 * g:32 * g + 32, 32 * p:32 * p + 32],
                            rhs=xt[32 * g:32 * g + 32,
                                   BSHARD * xcol:BSHARD * xcol + BSHARD],
                            start=st, stop=sp, skip_group_check=True,
                            tile_position=(32 * g, 32 * s))

                # out = sum of the 4 row-group copies + bias
                # (DVE can take at most one PSUM operand per instruction)
                t1 = ep.tile([128, NBANKS * BSHARD], f32, tag="eacc",
                             name=f"t1_{r}")
                nc.scalar.activation(t1[:, :], banks[0][:, :], AF.Identity,
                                     bias=0.0)
                nc.vector.tensor_tensor(t1[:, :], t1[:, :], banks[1][:, :],
                                        ADD)
                nc.vector.tensor_tensor(t1[:, :], t1[:, :], banks[2][:, :],
                                        ADD)
                nc.vector.tensor_tensor(t1[:, :], t1[:, :], banks[3][:, :],
                                        ADD)
                ot = opool.tile([128, NBANKS * BSHARD], f32, tag="ot",
                                name=f"ot{r}")
                for u in range(4):
                    nc.scalar.activation(
                        ot[:, BSHARD * u:BSHARD * (u + 1)],
                        t1[:, BSHARD * u:BSHARD * (u + 1)], AF.Identity,
                        bias=bias[:, 4 * r + u:4 * r + u + 1])
                nc.sync.dma_start(out_d.ap()[r, :, :], ot[:, :])

    nc.compile()
    return nc


# ---------------------------------------------------------------- entry

def _get_program(row_g, col_g):
    import concourse.mybir as mybir
    import ml_dtypes
    mode = _dt_mode()
    key = (row_g.tobytes(), col_g.tobytes(), mode)
    if key not in _prog_cache:
        sched, rounds = _plan(row_g, col_g)
        dt_w = mybir.dt.float32 if mode == "fp32" else mybir.dt.bfloat16
        np_dt = np.float32 if mode == "fp32" else ml_dtypes.bfloat16
        nc = _build(sched, dt_w)
        _prog_cache[key] = (nc, sched, rounds, np_dt)
    return _prog_cache[key]


def make_in_maps(inputs):
    """Host-side shard + pack.  Returns (nc, in_maps, rounds)."""
    row_g = np.asarray(inputs["row_g"])
    col_g = np.asarray(inputs["col_g"])
    nc, sched, rounds, np_dt = _get_program(row_g, col_g)

    x = np.asarray(inputs["x"], np.float32)
    wm = _pack_weights(inputs["weight_mean"], sched, np_dt)
    wl = _pack_weights(inputs["weight_log_var"], sched, np_dt)
    we = _pack_weights(inputs["eps_w"], sched, np_dt)
    bm = _pack_bias(np.asarray(inputs["b_mean"], np.float32), rounds)
    bl = _pack_bias(np.asarray(inputs["b_log_var"], np.float32), rounds)
    be = _pack_bias(np.asarray(inputs["eps_b"], np.float32), rounds)

    shared = {}
    for r in range(NROUNDS):
        shared[f"wm_{r}"] = wm[r]
        shared[f"wl_{r}"] = wl[r]
        shared[f"we_{r}"] = we[r]
    shared["bm_packed"] = bm
    shared["bl_packed"] = bl
    shared["be_packed"] = be

    in_maps = []
    for k in range(NCORES):
        m = dict(shared)
        m["x_packed"] = _pack_x(
            np.ascontiguousarray(x[:, k * BSHARD:(k + 1) * BSHARD]), np_dt)
        in_maps.append(m)
    return nc, in_maps, rounds


def kernel(**inputs):
    global LAST_PROFILE
    from concourse import bass_utils

    nc, in_maps, rounds = make_in_maps(inputs)
    trace = os.environ.get("BSL_TRACE", "0") == "1"
    res = bass_utils.run_bass_kernel_spmd(
        nc, in_maps, core_ids=list(range(NCORES)), trace=trace)
    LAST_PROFILE = {
        "exec_time_ns": res.exec_time_ns,
        "mean_exec_time_ns": res.mean_exec_time_ns,
        "max_exec_time_core_id": res.max_exec_time_core_id,
        "trace": (res.instructions_and_trace[1]
                  if res.instructions_and_trace else None),
    }
    out = np.zeros((G2 * A2, B), np.float32)
    for k in range(NCORES):
        out[:, k * BSHARD:(k + 1) * BSHARD] = _unpack_out(
            res.results[k]["out_packed"], rounds)
    return out, np.float32(0.0)


# revision 6
# speedup vs baseline: 1.6560x; 1.6560x over previous
"""Bayesian block-sparse linear layer (gnn message passing) on 8 Trainium2 cores.

out = segment_sum_e( v[e].T @ x_block[col_g[e]] ) + bias,
v[e] = eps_w[e] * exp(weight_log_var[e]) + weight_mean[e]   (32x32 blocks)

Strategy:
  * Batch-shard x across the 8 cores (128 columns each) -> one SPMD program.
  * The graph (row_g/col_g) is host-visible, so the program is specialized to
    it: every per-edge matmul is a 32x32 PE sub-array matmul (tile_position)
    so up to 16 edge-matmuls execute concurrently on the PE array.
  * All of x lives in SBUF in a block-permuted layout (block c at partition
    group c%4).  Edge weights are sampled on device (ACT exp + 2 DVE ops)
    from host-packed per-round layouts, then consumed as matmul lhsT.
  * Output blocks are processed in 16 rounds of 16 blocks; each block owns a
    [32, 512] PSUM strip = 4 private accumulator copies (one per PE row
    group) so concurrent sub-array matmuls never accumulate into the same
    PSUM element.  Evacuation sums the 4 copies and adds the sampled bias
    (ACT Identity with per-partition bias + 3 DVE adds).
"""

import os

import numpy as np

# problem dims (hardcoded per spec)
G1 = 256
G2 = 256
A1 = 32
A2 = 32
B = 1024
NCORES = 8
BSHARD = B // NCORES  # 128
NROUNDS = 16
BPR = 16    # blocks per round
NBANKS = 4  # PSUM banks per round
ZED = object()  # sentinel

LAST_PROFILE = None
_prog_cache = {}


def _dt_mode():
    return os.environ.get("BSL_DTYPE", "fp32")


# ---------------------------------------------------------------- host plan

def _plan(row_g, col_g):
    """Specialize the schedule to the graph."""
    E = len(row_g)
    blk = [[[] for _ in range(4)] for _ in range(G2)]
    for e in range(E):
        blk[int(row_g[e])][int(col_g[e]) % 4].append(e)
    cnts = np.array([[len(blk[q][g]) for g in range(4)] for q in range(G2)])

    # greedy bin-pack blocks into rounds, balancing per-partition-group load
    order = np.argsort(-cnts.sum(1), kind="stable")
    rounds = [[] for _ in range(NROUNDS)]
    load = np.zeros((NROUNDS, 4), np.int64)
    for q in order:
        best, bkey = None, None
        for r in range(NROUNDS):
            if len(rounds[r]) >= BPR:
                continue
            nl = load[r] + cnts[q]
            key = (int(nl.max()), int(nl.sum()))
            if best is None or key < bkey:
                best, bkey = r, key
        rounds[best].append(int(q))
        load[best] += cnts[q]

    blkmap = {}
    for r in range(NROUNDS):
        for idx, q in enumerate(rounds[r]):
            blkmap[q] = (r, idx)

    sched = []
    for r in range(NROUNDS):
        glists = [[] for _ in range(4)]
        for q in rounds[r]:
            for g in range(4):
                lst = blk[q][g]
                if lst:
                    for e in lst:
                        glists[g].append((e, q))
                else:
                    # region never written otherwise -> dummy zero matmul
                    glists[g].append((E, q))
        # round-robin by slot so consecutive same-group matmuls hit
        # different PE sub-arrays
        for g in range(4):
            byslot = [[], [], [], []]
            for e, q in glists[g]:
                byslot[blkmap[q][1] // 4].append((e, q))
            inter = []
            i = 0
            while any(byslot):
                sl = byslot[i % 4]
                if sl:
                    inter.append(sl.pop(0))
                i += 1
            glists[g] = inter
        L = max(len(x) for x in glists)
        q0 = rounds[r][0]
        for g in range(4):
            while len(glists[g]) < L:
                glists[g].append((E, q0))
        # start/stop flags per (row-group bank g, slot s): the first matmul
        # on tile (g, s) clears that bank's 2KB slot row, the last closes
        # the accumulation group.  Only tile (g, s) writes that region, so
        # the clear/accumulate ordering is the PE's own FIFO - race-free.
        first_pos, last_pos = {}, {}
        for p in range(L):
            for g in range(4):
                e, q = glists[g][p]
                s = blkmap[q][1] // 4
                if (g, s) not in first_pos:
                    first_pos[(g, s)] = p
                last_pos[(g, s)] = p
        entries = [[None] * L for _ in range(4)]
        widx = np.full((4, L), E, np.int64)
        for p in range(L):
            for g in range(4):
                e, q = glists[g][p]
                ridx = blkmap[q][1]
                s, u = ridx // 4, ridx % 4
                xcol = (int(col_g[e]) // 4) if e < E else 0
                entries[g][p] = (s, u, xcol,
                                 first_pos[(g, s)] == p,
                                 last_pos[(g, s)] == p)
                widx[g, p] = e
        sched.append({"L": L, "entries": entries, "widx": widx})
    return sched, rounds


# ---------------------------------------------------------------- host pack

def _pack_weights(w, sched, np_dt):
    w = np.asarray(w, np.float32).reshape(-1, A1, A2)
    wext = np.concatenate([w, np.zeros((1, A1, A2), np.float32)], 0)
    outs = []
    for sc in sched:
        t = wext[sc["widx"]]                       # [4, L, 32, 32]
        t = t.transpose(0, 2, 1, 3).reshape(128, 32 * sc["L"])
        outs.append(np.ascontiguousarray(t.astype(np_dt)))
    return outs


def _pack_x(xk, np_dt):  # xk [8192, BSHARD]
    t = xk.reshape(64, 4, 32, BSHARD).transpose(1, 2, 0, 3)
    return np.ascontiguousarray(t.reshape(128, 64 * BSHARD).astype(np_dt))


def _pack_bias(vec, rounds):  # vec [8192] fp32
    out = np.zeros((NROUNDS, NBANKS, 128), np.float32)
    for r in range(NROUNDS):
        for idx, q in enumerate(rounds[r]):
            s, bb = idx // 4, idx % 4
            out[r, bb, 32 * s:32 * s + 32] = vec[32 * q:32 * q + 32]
    return np.ascontiguousarray(
        out.transpose(2, 0, 1).reshape(128, NROUNDS * NBANKS))


def _unpack_out(op, rounds):  # op [NROUNDS, 128, 4*BSHARD] fp32
    t = op.reshape(NROUNDS, 4, 32, 4, BSHARD).transpose(0, 1, 3, 2, 4)
    res = np.zeros((G2, 32, BSHARD), np.float32)
    for r in range(NROUNDS):
        k = t[r].reshape(16, 32, BSHARD)
        for idx, q in enumerate(rounds[r]):
            res[q] = k[idx]
    return res.reshape(G2 * 32, BSHARD)


# ---------------------------------------------------------------- program

def _build(sched, dt_w):
    import concourse.bacc as bacc
    import concourse.mybir as mybir
    import concourse.tile as tile_mod

    nc = bacc.Bacc("TRN2", target_bir_lowering=False, debug=False,
                   num_devices=NCORES)
    f32 = mybir.dt.float32
    AF = mybir.ActivationFunctionType
    ADD = mybir.AluOpType.add
    MUL = mybir.AluOpType.mult

    x_d = nc.dram_tensor("x_packed", [128, 64 * BSHARD], dt_w,
                         kind="ExternalInput")
    wm_d = [nc.dram_tensor(f"wm_{r}", [128, 32 * sched[r]["L"]], dt_w,
                           kind="ExternalInput") for r in range(NROUNDS)]
    wl_d = [nc.dram_tensor(f"wl_{r}", [128, 32 * sched[r]["L"]], dt_w,
                           kind="ExternalInput") for r in range(NROUNDS)]
    we_d = [nc.dram_tensor(f"we_{r}", [128, 32 * sched[r]["L"]], dt_w,
                           kind="ExternalInput") for r in range(NROUNDS)]
    bm_d = nc.dram_tensor("bm_packed", [128, NROUNDS * NBANKS], f32,
                          kind="ExternalInput")
    bl_d = nc.dram_tensor("bl_packed", [128, NROUNDS * NBANKS], f32,
                          kind="ExternalInput")
    be_d = nc.dram_tensor("be_packed", [128, NROUNDS * NBANKS], f32,
                          kind="ExternalInput")
    out_d = nc.dram_tensor("out_packed", [NROUNDS, 128, NBANKS * BSHARD], f32,
                           kind="ExternalOutput")

    with tile_mod.TileContext(nc) as tc:
        with tc.tile_pool(name="xp", bufs=1) as xp, \
             tc.tile_pool(name="bp", bufs=1) as bp, \
             tc.tile_pool(name="wp", bufs=2) as wp, \
             tc.tile_pool(name="pp", bufs=8, space="PSUM") as pp, \
             tc.tile_pool(name="ep", bufs=8) as ep, \
             tc.tile_pool(name="opool", bufs=2) as opool:

            xt = xp.tile([128, 64 * BSHARD], dt_w, name="xt")
            nc.sync.dma_start(xt[:, :], x_d.ap())

            bmt = bp.tile([128, NROUNDS * NBANKS], f32, name="bmt")
            nc.sync.dma_start(bmt[:, :], bm_d.ap())
            blt = bp.tile([128, NROUNDS * NBANKS], f32, name="blt")
            nc.sync.dma_start(blt[:, :], bl_d.ap())
            bet = bp.tile([128, NROUNDS * NBANKS], f32, name="bet")
            nc.sync.dma_start(bet[:, :], be_d.ap())
            bias = bp.tile([128, NROUNDS * NBANKS], f32, name="bias")
            nc.scalar.activation(bias[:, :], blt[:, :], AF.Exp)
            nc.vector.tensor_tensor(bias[:, :], bias[:, :], bet[:, :], MUL)
            nc.vector.tensor_tensor(bias[:, :], bias[:, :], bmt[:, :], ADD)

            for r in range(NROUNDS):
                L = sched[r]["L"]
                W = 32 * L
                wmt = wp.tile([128, W], dt_w, tag="wm", name=f"wm_t{r}")
                nc.sync.dma_start(wmt[:, :], wm_d[r].ap())
                wlt = wp.tile([128, W], dt_w, tag="wl", name=f"wl_t{r}")
                nc.sync.dma_start(wlt[:, :], wl_d[r].ap())
                wet = wp.tile([128, W], dt_w, tag="we", name=f"we_t{r}")
                nc.sync.dma_start(wet[:, :], we_d[r].ap())

                # v = eps * exp(log_var) + mean
                nc.scalar.activation(wlt[:, :], wlt[:, :], AF.Exp)
                nc.vector.tensor_tensor(wet[:, :], wet[:, :], wlt[:, :], MUL)
                nc.vector.tensor_tensor(wet[:, :], wet[:, :], wmt[:, :], ADD)

                # bank g is private to PE row-group g: no two sub-array
                # matmuls ever write the same (bank, partition-range).
                banks = [pp.tile([128, NBANKS * BSHARD], f32, tag="bank",
                                 name=f"bank{r}_{b}") for b in range(4)]
                ents = sched[r]["entries"]
                for p in range(L):
                    for g in range(4):
                        s, u, xcol, st, sp = ents[g][p]
                        nc.tensor.matmul(
                            banks[g][32 * s:32 * s + 32,
                                     BSHARD * u:BSHARD * u + BSHARD],
                            lhsT=wet[32 * g:32 * g + 32, 32 * p:32 * p + 32],
                            rhs=xt[32 * g:32 * g + 32,
                                   BSHARD * xcol:BSHARD * xcol + BSHARD],
                            start=st, stop=sp, skip_group_check=True,
                            tile_position=(32 * g, 32 * s))

                # out = sum of the 4 row-group copies + bias
                # (DVE can take at most one PSUM operand per instruction)
                t1 = ep.tile([128, NBANKS * BSHARD], f32, tag="eacc",
                             name=f"t1_{r}")
                nc.scalar.activation(t1[:, :], banks[0][:, :], AF.Identity,
                                     bias=0.0)
                nc.vector.tensor_tensor(t1[:, :], t1[:, :], banks[1][:, :],
                                        ADD)
                nc.vector.tensor_tensor(t1[:, :], t1[:, :], banks[2][:, :],
                                        ADD)
                nc.vector.tensor_tensor(t1[:, :], t1[:, :], banks[3][:, :],
                                        ADD)
                ot = opool.tile([128, NBANKS * BSHARD], f32, tag="ot",
                                name=f"ot{r}")
                for u in range(4):
                    nc.scalar.activation(
                        ot[:, BSHARD * u:BSHARD * (u + 1)],
                        t1[:, BSHARD * u:BSHARD * (u + 1)], AF.Identity,
                        bias=bias[:, 4 * r + u:4 * r + u + 1])
                nc.sync.dma_start(out_d.ap()[r, :, :], ot[:, :])

    nc.compile()
    return nc


# ---------------------------------------------------------------- entry

def _get_program(row_g, col_g):
    import concourse.mybir as mybir
    import ml_dtypes
    mode = _dt_mode()
    key = (row_g.tobytes(), col_g.tobytes(), mode)
    if key not in _prog_cache:
        sched, rounds = _plan(row_g, col_g)
        dt_w = mybir.dt.float32 if mode == "fp32" else mybir.dt.bfloat16
        np_dt = np.float32 if mode == "fp32" else ml_dtypes.bfloat16
        nc = _build(sched, dt_w)
        _prog_cache[key] = (nc, sched, rounds, np_dt)
    return _prog_cache[key]


def make_in_maps(inputs):
    """Host-side shard + pack.  Returns (nc, in_maps, rounds)."""
    row_g = np.asarray(inputs["row_g"])
    col_g = np.asarray(inputs["col_g"])
    nc, sched, rounds, np_dt = _get_program(row_g, col_g)

    x = np.asarray(inputs["x"], np.float32)
    wm = _pack_weights(inputs["weight_mean"], sched, np_dt)
    wl = _pack_weights(inputs["weight_log_var"], sched, np_dt)
    we = _pack_weights(inputs["eps_w"], sched, np_dt)
    bm = _pack_bias(np.asarray(inputs["b_mean"], np.float32), rounds)
    bl = _pack_bias(np.asarray(inputs["b_log_var"], np.float32), rounds)
    be = _pack_bias(np.asarray(inputs["eps_b"], np.float32), rounds)

    shared = {}
    for r in range(NROUNDS):
        shared[f"wm_{r}"] = wm[r]
        shared[f"wl_{r}"] = wl[r]
        shared[f"we_{r}"] = we[r]
    shared["bm_packed"] = bm
    shared["bl_packed"] = bl
    shared["be_packed"] = be

    in_maps = []
    for k in range(NCORES):
        m = dict(shared)
        m["x_packed"] = _pack_x(
            np.ascontiguousarray(x[:, k * BSHARD:(k + 1) * BSHARD]), np_dt)
        in_maps.append(m)
    return nc, in_maps, rounds


def kernel(**inputs):
    global LAST_PROFILE
    from concourse import bass_utils

    nc, in_maps, rounds = make_in_maps(inputs)
    trace = os.environ.get("BSL_TRACE", "0") == "1"
    res = bass_utils.run_bass_kernel_spmd(
        nc, in_maps, core_ids=list(range(NCORES)), trace=trace)
    LAST_PROFILE = {
        "exec_time_ns": res.exec_time_ns,
        "mean_exec_time_ns": res.mean_exec_time_ns,
        "max_exec_time_core_id": res.max_exec_time_core_id,
        "trace": (res.instructions_and_trace[1]
                  if res.instructions_and_trace else None),
        "insts": (res.instructions_and_trace[0]
                  if res.instructions_and_trace else None),
    }
    out = np.zeros((G2 * A2, B), np.float32)
    for k in range(NCORES):
        out[:, k * BSHARD:(k + 1) * BSHARD] = _unpack_out(
            res.results[k]["out_packed"], rounds)
    return out, np.float32(0.0)
